# revision 29
# baseline (speedup 1.0000x reference)
"""Trainium2 Bass kernel for nn_Attention_80960133530355.

Math per (t,b) pair (A=64 agents, N=128 features, H=8 hidden):
    Q = X @ Wq + bq                  (64, 8)
    K = X @ Wk + bk                  (64, 8)
    Kr = K.reshape(8, 64)            # reshape, NOT transpose
    att = softmax(Q @ Kr, axis=-1)   (64, 64)
    out = att with diagonal removed  (64, 63)

Sharding: data-parallel over T (512 -> 64 per core), 8 cores, no collectives.

v6 design (all DMAs full-bandwidth, no on-chip transposes, no DMA fold):
  * Host feeds X^T (bf16) in a block-contiguous layout [128n, blk, g, e, a]
    so each block load is 128 descriptors x 8KB.
  * Key identity: att[a, 8p+q] = sum_m Qexp[m, a] * rhsD[m, 8p+q] where
      Qexp[m, a]   = Q[a, m//8]              (expanded Q, m = 0..63)
      rhsD[m, p q] = K[m, q] * [p == m%8]    (diagonal-scattered K)
    Qexp comes FREE from the projection by using expanded weights
    Wq_exp[:, m] = Wq[:, m//8] (duplicated in cols 64:128 so the e=1 pair
    lands on partitions 64:128).  att for the two pairs of a group runs as
    two 64x64 quadrant matmuls at tile positions (0,0) and (64,64).
  * K is produced by tiny per-group matmuls lhsT=X^T_g, rhs=Wk (ap=8) with
    a ones-row bias-priming matmul; rhsD by a Pool broadcast masked mul
    (GPSIMD cannot touch PSUM on the real compiler, so Pool only gets
    SBUF work; ACT/DVE split the PSUM->SBUF casts ~2.5:5.5).
  * Attention is emitted 2 quarters behind the rhs pipeline so the
    in-order PE never stalls on the k2->copy->mask chain.
  * Device computes exp(att); the host normalizes rows, reorders, and
    gathers off-diagonal columns while unsharding (same class of host
    work as the baseline's dtype cast + gather).
"""

import sys

import numpy as np

sys.path.insert(0, "/opt/trn_rl_repo")

import concourse.bass as bass
import concourse.bacc as bacc_mod
import concourse.mybir as mybir
from concourse.bass_utils import run_bass_kernel_spmd
from concourse.tile import TileContext

F32 = mybir.dt.float32
BF16 = mybir.dt.bfloat16

T, B, A, N, H = 512, 32, 64, 128, 8
NCORES = 8
T_SH = T // NCORES            # 64 T-rows per core
PAIRS = T_SH * B              # 2048 pairs per core
G = 8                         # groups (2 pairs each) per sub-block
SG = 32                       # groups per block
NSUB = SG // G                # 4 sub-blocks per block
NHALF = 2                     # half-blocks (16 groups) per block
BLOCK_PAIRS = 2 * SG          # 64 pairs per block
NBLK = PAIRS // BLOCK_PAIRS   # 32 blocks
AM1 = A - 1


def build_kernel(nblk=NBLK):
    nc = bacc_mod.Bacc(target_bir_lowering=False)

    x = nc.declare_dram_parameter("x", [128, NBLK * SG * 2 * A], BF16,
                                  isOutput=False)
    # packed bf16 constants: wcomb(128) | wk(8) | maskq(64) -> [128, 200]
    cpak = nc.declare_dram_parameter("cpak", [128, 200], BF16, isOutput=False)
    # row constants: ones(128) | bkrep(128) -> [1, 256]
    rpak = nc.declare_dram_parameter("rpak", [1, 256], BF16, isOutput=False)
    bvec = nc.declare_dram_parameter("bvec", [128, 1], F32, isOutput=False)
    out_es = nc.declare_dram_parameter("out_es", [128, NBLK * SG * A],
                                       BF16, isOutput=True)

    x_v = x.rearrange("p (blk f) -> p blk f", blk=NBLK)
    oe_v = out_es.rearrange("p (blk hb f) -> p blk hb f", blk=NBLK, hb=NHALF)

    with TileContext(nc) as tc:
        with (
            tc.tile_pool(name="const", bufs=1) as cpool,
            tc.tile_pool(name="xin", bufs=5) as xpool,
            tc.tile_pool(name="q", bufs=14) as qpool,
            tc.tile_pool(name="k2", bufs=4) as k2pool,
            tc.tile_pool(name="rhs", bufs=7) as rpool,
            tc.tile_pool(name="exp", bufs=5) as epool,
            tc.tile_pool(name="ps_pj", bufs=2, space="PSUM") as ps_pj,
            tc.tile_pool(name="ps_at", bufs=2, space="PSUM") as ps_at,
            tc.tile_pool(name="ps_k2", bufs=2, space="PSUM") as ps_k2,
        ):
            cp_sb = cpool.tile([128, 200], BF16, tag="cpak")
            rp_sb = cpool.tile([1, 256], BF16, tag="rpak")
            b_sb = cpool.tile([128, 1], F32, tag="b")

            w_sb = cp_sb[:, 0:128]
            wk_sb = cp_sb[:, 128:136]
            mq_sb = cp_sb[:, 136:200].rearrange("p (a b) -> p a b", a=H)
            ones_sb = rp_sb[:, 0:128]
            bk_sb = rp_sb[:, 128:256].rearrange("o (g q) -> o g q", g=2 * G)

            loaded = {}

            def _emit_load(b):
                if b >= nblk or b in loaded:
                    return
                t = xpool.tile([128, SG, 2 * A], BF16, tag="x")
                nc.sync.dma_start(
                    out=t[:, :, :],
                    in_=x_v[:, b, :].rearrange("p (g f) -> p g f", g=SG),
                )
                loaded[b] = t

            _emit_load(0)
            nc.sync.dma_start(out=cp_sb[:, :], in_=cpak[:, :])
            nc.sync.dma_start(out=rp_sb[:, :], in_=rpak[:, :])
            nc.sync.dma_start(out=b_sb[:, :], in_=bvec[:, :])
            _emit_load(1)
            ncast = 0
            att_q = []      # deferred quarters: (blk, hb, q, q_subs, rhs_v, es)

            def _emit_att(item):
                blk_, hb_, q_, q_subs, rhs_v, es_sb = item
                at_ps = ps_at.tile([128, 8, A], F32, tag="at")
                r0 = hb_ * 16 + q_ * 8
                for gq in range(8):
                    g_abs = r0 + gq
                    q_sb_g = q_subs[g_abs // G]
                    for e in range(2):
                        p0 = 64 * e
                        nc.tensor.matmul(
                            at_ps[p0:p0 + 64, gq:gq + 1, :],
                            q_sb_g[p0:p0 + 64, g_abs % G, :],
                            rhs_v[:, gq:gq + 1, :][p0:p0 + 64],
                            start=(gq == 0),
                            stop=(gq == 7),
                            skip_group_check=not (e == 0 and gq in (0, 7)),
                            tile_position=(p0, p0),
                        )
                nc.scalar.activation(
                    es_sb[:, q_ * 8:q_ * 8 + 8, :], at_ps[:, :, :],
                    mybir.ActivationFunctionType.Exp,
                )
                if q_ == 1:
                    nc.sync.dma_start(
                        out=oe_v[:, blk_, hb_, :].rearrange(
                            "p (g a) -> p g a", g=16),
                        in_=es_sb[:, :, :],
                    )

            for blk in range(nblk):
                _emit_load(blk + 1)
                _emit_load(blk + 2)
                xt = loaded.pop(blk)
                xt_flat = xt[:, :, :].rearrange("p g f -> p (g f)")

                k2_sb = k2pool.tile([128, SG, H], BF16, tag="k2")
                q_blk = []
                for s in range(NSUB):
                    # ---- projection: expanded-Q (dup halves) ----
                    pj = ps_pj.tile([128, 2, 512], F32, tag="pj")
                    for h in range(2):
                        nc.tensor.matmul(
                            pj[:, h:h + 1, :],
                            w_sb[:, :],
                            xt_flat[:, s * 1024 + h * 512:s * 1024 + (h + 1) * 512],
                            start=True,
                            stop=True,
                        )
                    # ---- cast+bias: Qexp halves to SBUF (ACT/DVE split) ----
                    q_sb = qpool.tile([128, G, A], BF16, tag="q")
                    q_blk.append(q_sb)
                    pj_v = pj[:, :, :].rearrange("p h f -> p (h f)").rearrange(
                        "p (g e a) -> p g e a", g=G, e=2)
                    for e in range(2):
                        dst = q_sb[64 * e:64 * e + 64, :, :]
                        src = pj_v[64 * e:64 * e + 64, :, e, :]
                        bias = b_sb[64 * e:64 * e + 64, :]
                        # per block: 2 casts on ACT, 5 on DVE, 1 split
                        ci = ncast % 8
                        if ci in (0, 4):
                            nc.scalar.activation(
                                dst, src,
                                mybir.ActivationFunctionType.Identity,
                                bias=bias,
                            )
                        elif ci == 2:
                            nc.scalar.activation(
                                dst[:, 0:4, :], src[:, 0:4, :],
                                mybir.ActivationFunctionType.Identity,
                                bias=bias,
                            )
                            nc.vector.tensor_scalar_add(
                                dst[:, 4:8, :], src[:, 4:8, :], bias)
                        else:
                            nc.vector.tensor_scalar_add(dst, src, bias)
                        ncast += 1

                    if s % 2 == 0:
                        continue
                    # ---- half-block: rhs pipeline by quarters; defer att ----
                    hb = s // 2
                    es_sb = epool.tile([128, 16, A], BF16, tag="exp")
                    for q in range(2):
                        r0 = hb * 16 + q * 8
                        # K natural for these 8 groups (bias-primed)
                        k2_ps = ps_k2.tile([128, 8, H], F32, tag="k2p")
                        nc.tensor.matmul(
                            k2_ps[:, :, :], ones_sb[:, :],
                            bk_sb[:, 0:8, :],
                            start=True, stop=False, skip_group_check=False,
                        )
                        for g in range(8):
                            nc.tensor.matmul(
                                k2_ps[:, g:g + 1, :],
                                xt[:, r0 + g, :],
                                wk_sb[:, :],
                                start=False,
                                stop=(g == 7),
                                skip_group_check=(g != 7),
                            )
                        nc.vector.tensor_copy(k2_sb[:, r0:r0 + 8, :],
                                              k2_ps[:, :, :])
                        # diag-scatter rhs (Pool)
                        rhs = rpool.tile([128, 8, H, H], BF16, tag="rhs")
                        k2b = k2_sb[:, r0:r0 + 8, :].unsqueeze(2).broadcast_to(
                            (128, 8, H, H))
                        mqb = mq_sb.unsqueeze(1).broadcast_to((128, 8, H, H))
                        nc.gpsimd.tensor_tensor(
                            rhs[:, :, :, :], k2b, mqb, mybir.AluOpType.mult)
                        rhs_v = rhs[:, :, :, :].rearrange(
                            "p g x y -> p g (x y)")
                        att_q.append((blk, hb, q, q_blk, rhs_v, es_sb))
                        if len(att_q) > 2:
                            _emit_att(att_q.pop(0))
            while att_q:
                _emit_att(att_q.pop(0))

    return nc


def _host_constants(Wq, bq, Wk, bk):
    import ml_dtypes

    bf = ml_dtypes.bfloat16
    cpak = np.empty((128, 200), dtype=bf)
    wq_exp = Wq[:, np.arange(64) // 8]          # (N, 64)
    cpak[:, 0:64] = wq_exp
    cpak[:, 64:128] = wq_exp
    cpak[:, 128:136] = Wk
    m = np.arange(128) % 8
    maskq = (np.arange(8)[None, :, None] == m[:, None, None])
    cpak[:, 136:200] = np.broadcast_to(maskq, (128, 8, 8)).reshape(128, 64)
    rpak = np.empty((1, 256), dtype=bf)
    rpak[0, 0:128] = 1.0
    rpak[0, 128:256] = np.tile(bk, 2 * G)
    bvec = bq[(np.arange(128) % 64) // 8].astype(np.float32).reshape(128, 1)
    return dict(cpak=cpak, rpak=rpak, bvec=bvec)


_OFFDIAG_COLS = None


def _offdiag_cols():
    global _OFFDIAG_COLS
    if _OFFDIAG_COLS is None:
        idx = np.arange(A)
        _OFFDIAG_COLS = np.stack(
            [np.delete(idx, i) for i in range(A)], axis=0)
    return _OFFDIAG_COLS


def _cache_nc(_cache={}):
    if "nc" not in _cache:
        nc = build_kernel()
        nc.finalize()
        _cache["nc"] = nc
    return _cache["nc"]


def host_pack_x(agent_state):
    """x^T per core: [core, n, blk, g, e, a] contiguous bf16."""
    import ml_dtypes

    xb = agent_state.astype(ml_dtypes.bfloat16)
    xb = xb.reshape(NCORES, NBLK, SG, 2, A, N)
    xb = np.ascontiguousarray(xb.transpose(0, 5, 1, 2, 3, 4))
    return xb.reshape(NCORES, 128, NBLK * SG * 2 * A)


def host_unpack(es):
    """[128, NBLK*SG*64] bf16 exp -> (T_SH, B, A, A-1) f32 softmax w/o diag."""
    es = np.asarray(es).astype(np.float32).reshape(128, NBLK, SG, A)
    soft = es / es.sum(axis=-1, keepdims=True)
    soft = soft.reshape(2, A, NBLK, SG, A).transpose(2, 3, 0, 1, 4)
    soft = soft.reshape(T_SH, B, A, A)
    cols = _offdiag_cols()
    return np.take_along_axis(soft, cols[None, None, :, :], axis=-1)


def kernel(agent_state, Wq, bq, Wk, bk):
    agent_state = np.asarray(agent_state, dtype=np.float32)
    Wq = np.asarray(Wq, dtype=np.float32)
    bq = np.asarray(bq, dtype=np.float32)
    Wk = np.asarray(Wk, dtype=np.float32)
    bk = np.asarray(bk, dtype=np.float32)

    nc = _cache_nc()
    consts = _host_constants(Wq, bq, Wk, bk)
    xb = host_pack_x(agent_state)

    in_maps = []
    for c in range(NCORES):
        m = {"x": xb[c]}
        m.update(consts)
        in_maps.append(m)

    res = run_bass_kernel_spmd(nc, in_maps, core_ids=list(range(NCORES)))
    outs = [host_unpack(r["out_es"]) for r in res.results]
    return np.concatenate(outs, axis=0)


if __name__ == "__main__":
    rng = np.random.default_rng(0)
    xs = rng.standard_normal((T, B, A, N), dtype=np.float32)
    s = 1 / np.sqrt(N)
    r = kernel(
        agent_state=xs,
        Wq=rng.uniform(-s, s, (N, H)).astype(np.float32),
        bq=rng.uniform(-s, s, (H,)).astype(np.float32),
        Wk=rng.uniform(-s, s, (N, H)).astype(np.float32),
        bk=rng.uniform(-s, s, (H,)).astype(np.float32),
    )
    print(r.shape, r.dtype)
